# revision 7
# baseline (speedup 1.0000x reference)
"""Trainium2 Bass kernel for nn_Block2x2DiagProductRectangular.

The reference applies 10 butterfly stages (fixed 2x2 factor matrices) along the
feature axis of x [16384, 1024], then adds a bias. Since the factors are fixed
inputs, the whole chain is one dense linear map: out = x @ M + bias with
M = product of the butterfly stage matrices (1024x1024).

Strategy:
  - Host: build M in float64 from abcd_list, cast to fp32.
  - Shard batch across 8 NeuronCores (2048 rows each).
  - Host pre-transposes each x shard so the device needs no on-chip transposes:
    the PE matmul stationary operand is x^T tiles [K=128 feat, M=128 batch],
    moving operand is M row-blocks [128, 512] resident in SBUF, accumulating
    out tiles [128 batch, 1024 feat] in PSUM over 8 K-tiles (fp32r, 1 cyc/row).
  - DVE drains PSUM with a fused bias add; stores go out on the second HWDGE
    queue (nc.scalar) so they don't serialize behind loads on nc.sync.
  - Group 0's inputs are DMA'd in per-K chunks so the first matmul only waits
    for ~0.75 MB; later groups arrive as single prefetched transfers.
  - A few dummy matmuls on scratch data warm the PE HAM clock gate during the
    initial DMA window.
"""

import numpy as np

import concourse.bass as bass
import concourse.mybir as mybir
import concourse.tile as tile
from concourse import bacc
from concourse.bass_utils import run_bass_kernel_spmd

BATCH = 16384
N = 1024
P = 128
NCORES = 8
ROWS_PER_CORE = BATCH // NCORES          # 2048
GROUPS = 4                               # batch groups per core (512 rows each)
GROUP_ROWS = ROWS_PER_CORE // GROUPS     # 512
BT_PER_GROUP = GROUP_ROWS // P           # 4
SUBGROUP_BT = 2                          # batch tiles accumulated concurrently
KO = N // P                              # 8 k-tiles
WARMUP_MM = 8


def _build_dense_matrix(abcd_list):
    """Dense M (float64) such that reference(x) == x @ M + bias."""
    out = np.eye(N, dtype=np.float64)
    for abcd in abcd_list[::-1]:
        half = abcd.shape[-1]
        a = np.asarray(abcd, dtype=np.float64)[0]          # [2, 2, half]
        y = out.reshape(N, -1, 2, half)
        y = np.einsum('ikj,bgkj->bgij', a, y)
        out = y.reshape(N, N)
    return out


def _build_bass():
    nc = bacc.Bacc(None, target_bir_lowering=False, debug=False)
    xt_d = nc.dram_tensor(
        "xt", (GROUPS, P, KO, GROUP_ROWS), mybir.dt.float32r, kind="ExternalInput"
    )
    wt_d = nc.dram_tensor("wt", (KO, P, N), mybir.dt.float32r, kind="ExternalInput")
    bias_d = nc.dram_tensor("bias_bc", (P, N), mybir.dt.float32, kind="ExternalInput")
    out_d = nc.dram_tensor(
        "out", (ROWS_PER_CORE, N), mybir.dt.float32, kind="ExternalOutput"
    )

    with tile.TileContext(nc) as tc:
        with (
            tc.tile_pool(name="const", bufs=1) as const_pool,
            tc.tile_pool(name="xt", bufs=4) as xt_pool,
            tc.tile_pool(name="outs", bufs=3) as out_pool,
            tc.tile_pool(name="psum", bufs=3, space="PSUM") as psum_pool,
            tc.tile_pool(name="warm", bufs=1, space="PSUM") as warm_psum_pool,
        ):
            # PE warm-up: dummy matmuls on scratch data, no DMA dependency, so
            # the HAM clock gate opens during the initial load window.
            warm_sb = const_pool.tile([P, 512], mybir.dt.float32)
            nc.gpsimd.memset(warm_sb[:], 0.0)
            warm_ps = warm_psum_pool.tile([P, 512], mybir.dt.float32)
            for _ in range(WARMUP_MM):
                nc.tensor.matmul(
                    warm_ps[:], warm_sb[:, :P], warm_sb[:],
                    start=True, stop=True,
                )

            wt_sb = const_pool.tile([P, KO, N], mybir.dt.float32r)
            bias_sb = const_pool.tile([P, N], mybir.dt.float32)

            xt_tiles = []
            for g in range(GROUPS):
                xt_sb = xt_pool.tile([P, KO, GROUP_ROWS], mybir.dt.float32r,
                                     name=f"xt_sb_{g}", tag="xt_sb")
                xt_tiles.append(xt_sb)
                if g == 0:
                    # chunked with the W tiles so the first matmul starts early
                    for ko in range(KO):
                        nc.sync.dma_start(wt_sb[:, ko, :], wt_d[ko])
                        nc.sync.dma_start(xt_sb[:, ko, :], xt_d[g][:, ko, :])
                    nc.scalar.dma_start(bias_sb[:], bias_d[:])
                else:
                    nc.sync.dma_start(xt_sb[:], xt_d[g])

            for g in range(GROUPS):
                xt_sb = xt_tiles[g]
                for sg in range(BT_PER_GROUP // SUBGROUP_BT):
                    pss = [
                        psum_pool.tile([P, N], mybir.dt.float32, name="ps_acc")
                        for _ in range(SUBGROUP_BT)
                    ]
                    for ko in range(KO):
                        for i in range(SUBGROUP_BT):
                            bt = sg * SUBGROUP_BT + i
                            lhsT = xt_sb[:, ko, bt * P:(bt + 1) * P]
                            for h in range(N // 512):
                                nc.tensor.matmul(
                                    pss[i][:, h * 512:(h + 1) * 512],
                                    lhsT,
                                    wt_sb[:, ko, h * 512:(h + 1) * 512],
                                    start=(ko == 0),
                                    stop=(ko == KO - 1),
                                )
                    for i in range(SUBGROUP_BT):
                        bt = sg * SUBGROUP_BT + i
                        out_sb = out_pool.tile([P, N], mybir.dt.float32)
                        nc.vector.tensor_add(
                            out=out_sb[:], in0=pss[i][:], in1=bias_sb[:]
                        )
                        row0 = g * GROUP_ROWS + bt * P
                        nc.scalar.dma_start(out_d[row0:row0 + P, :], out_sb[:])

    nc.compile()
    return nc


def kernel(x, abcd_list, bias, _trace=False):
    x = np.ascontiguousarray(np.asarray(x, dtype=np.float32))
    bias = np.asarray(bias, dtype=np.float32)

    M = _build_dense_matrix(abcd_list).astype(np.float32)
    wt3 = np.ascontiguousarray(M.reshape(KO, P, N))       # [ko, p, n]
    bias_bc = np.ascontiguousarray(np.broadcast_to(bias[None, :], (P, N)))

    nc = _build_bass()

    in_maps = []
    for c in range(NCORES):
        xs = x[c * ROWS_PER_CORE:(c + 1) * ROWS_PER_CORE]
        # xt4[g, p, ko, b] = xs[g*512 + b, ko*128 + p]
        xt4 = np.ascontiguousarray(
            xs.reshape(GROUPS, GROUP_ROWS, KO, P).transpose(0, 3, 2, 1)
        )
        in_maps.append({"xt": xt4, "wt": wt3, "bias_bc": bias_bc})

    res = run_bass_kernel_spmd(
        nc, in_maps, core_ids=list(range(NCORES)), trace=_trace
    )
    out = np.concatenate([r["out"] for r in res.results], axis=0)
    if _trace:
        kernel.last_results = res
    return out


# revision 9
# speedup vs baseline: 1.1351x; 1.1351x over previous
"""Trainium2 Bass kernel for nn_Block2x2DiagProductRectangular.

The reference applies 10 butterfly stages (fixed 2x2 factor matrices) along the
feature axis of x [16384, 1024], then adds a bias. Since the factors are fixed
inputs, the whole chain is one dense linear map: out = x @ M + bias with
M = product of the butterfly stage matrices (1024x1024).

Strategy:
  - Host: build M in float64 from abcd_list, cast to fp32.
  - Shard batch across 8 NeuronCores (2048 rows each).
  - Host pre-transposes each x shard so the device needs no on-chip transposes:
    the PE matmul stationary operand is x^T tiles [K=128 feat, M=128 batch],
    moving operand is M row-blocks [128, 512] resident in SBUF, accumulating
    out tiles [128 batch, 1024 feat] in PSUM over 8 K-tiles (fp32r, 1 cyc/row).
  - DVE drains PSUM with a fused bias add; stores go out on the second HWDGE
    queue (nc.scalar) so they don't serialize behind loads on nc.sync.
  - Group 0's inputs are DMA'd in per-K chunks so the first matmul only waits
    for ~0.75 MB; later groups arrive as single prefetched transfers.
  - A few dummy matmuls on scratch data warm the PE HAM clock gate during the
    initial DMA window.
"""

import numpy as np

import concourse.bass as bass
import concourse.mybir as mybir
import concourse.tile as tile
from concourse import bacc
from concourse.bass_utils import run_bass_kernel_spmd

BATCH = 16384
N = 1024
P = 128
NCORES = 8
ROWS_PER_CORE = BATCH // NCORES          # 2048
GROUPS = 4                               # batch groups per core (512 rows each)
GROUP_ROWS = ROWS_PER_CORE // GROUPS     # 512
BT_PER_GROUP = GROUP_ROWS // P           # 4
SUBGROUP_BT = 2                          # batch tiles accumulated concurrently
KO = N // P                              # 8 k-tiles
WARMUP_MM = 8


def _build_dense_matrix(abcd_list):
    """Dense M (float64) such that reference(x) == x @ M + bias."""
    out = np.eye(N, dtype=np.float64)
    for abcd in abcd_list[::-1]:
        half = abcd.shape[-1]
        a = np.asarray(abcd, dtype=np.float64)[0]          # [2, 2, half]
        y = out.reshape(N, -1, 2, half)
        y = np.einsum('ikj,bgkj->bgij', a, y)
        out = y.reshape(N, N)
    return out


def _build_bass():
    nc = bacc.Bacc(None, target_bir_lowering=False, debug=False)
    xt_d = nc.dram_tensor(
        "xt", (GROUPS, P, KO, GROUP_ROWS), mybir.dt.float32r, kind="ExternalInput"
    )
    wt_d = nc.dram_tensor("wt", (KO, P, N), mybir.dt.float32r, kind="ExternalInput")
    bias_d = nc.dram_tensor("bias_bc", (P, N), mybir.dt.float32, kind="ExternalInput")
    out_d = nc.dram_tensor(
        "out", (ROWS_PER_CORE, N), mybir.dt.float32, kind="ExternalOutput"
    )

    with tile.TileContext(nc) as tc:
        with (
            tc.tile_pool(name="const", bufs=1) as const_pool,
            tc.tile_pool(name="xt", bufs=4) as xt_pool,
            tc.tile_pool(name="outs", bufs=6) as out_pool,
            tc.tile_pool(name="psum", bufs=3, space="PSUM") as psum_pool,
            tc.tile_pool(name="warm", bufs=1, space="PSUM") as warm_psum_pool,
        ):
            # PE warm-up: dummy matmuls on scratch data, no DMA dependency, so
            # the HAM clock gate opens during the initial load window.
            warm_sb = const_pool.tile([P, 512], mybir.dt.float32)
            nc.gpsimd.memset(warm_sb[:], 0.0)
            warm_ps = warm_psum_pool.tile([P, 512], mybir.dt.float32)
            for _ in range(WARMUP_MM):
                nc.tensor.matmul(
                    warm_ps[:], warm_sb[:, :P], warm_sb[:],
                    start=True, stop=True,
                )

            wt_sb = const_pool.tile([P, KO, N], mybir.dt.float32r)
            bias_sb = const_pool.tile([P, N], mybir.dt.float32)

            xt_tiles = []
            for g in range(GROUPS):
                xt_sb = xt_pool.tile([P, KO, GROUP_ROWS], mybir.dt.float32r,
                                     name=f"xt_sb_{g}", tag="xt_sb")
                xt_tiles.append(xt_sb)
                if g == 0:
                    # chunked with the W tiles so the first matmul starts early
                    for ko in range(KO):
                        nc.sync.dma_start(wt_sb[:, ko, :], wt_d[ko])
                        nc.sync.dma_start(xt_sb[:, ko, :], xt_d[g][:, ko, :])
                    nc.scalar.dma_start(bias_sb[:], bias_d[:])
                else:
                    nc.sync.dma_start(xt_sb[:], xt_d[g])

            for g in range(GROUPS):
                xt_sb = xt_tiles[g]
                for bt in range(BT_PER_GROUP):
                    ps = psum_pool.tile([P, N], mybir.dt.float32, name="ps_acc")
                    for ko in range(KO):
                        lhsT = xt_sb[:, ko, bt * P:(bt + 1) * P]
                        for h in range(N // 512):
                            nc.tensor.matmul(
                                ps[:, h * 512:(h + 1) * 512],
                                lhsT,
                                wt_sb[:, ko, h * 512:(h + 1) * 512],
                                start=(ko == 0),
                                stop=(ko == KO - 1),
                            )
                    out_sb = out_pool.tile([P, N], mybir.dt.float32)
                    nc.vector.tensor_add(
                        out=out_sb[:], in0=ps[:], in1=bias_sb[:]
                    )
                    row0 = g * GROUP_ROWS + bt * P
                    nc.scalar.dma_start(out_d[row0:row0 + P, :], out_sb[:])

    nc.compile()
    return nc


def kernel(x, abcd_list, bias, _trace=False):
    x = np.ascontiguousarray(np.asarray(x, dtype=np.float32))
    bias = np.asarray(bias, dtype=np.float32)

    M = _build_dense_matrix(abcd_list).astype(np.float32)
    wt3 = np.ascontiguousarray(M.reshape(KO, P, N))       # [ko, p, n]
    bias_bc = np.ascontiguousarray(np.broadcast_to(bias[None, :], (P, N)))

    nc = _build_bass()

    in_maps = []
    for c in range(NCORES):
        xs = x[c * ROWS_PER_CORE:(c + 1) * ROWS_PER_CORE]
        # xt4[g, p, ko, b] = xs[g*512 + b, ko*128 + p]
        xt4 = np.ascontiguousarray(
            xs.reshape(GROUPS, GROUP_ROWS, KO, P).transpose(0, 3, 2, 1)
        )
        in_maps.append({"xt": xt4, "wt": wt3, "bias_bc": bias_bc})

    res = run_bass_kernel_spmd(
        nc, in_maps, core_ids=list(range(NCORES)), trace=_trace
    )
    out = np.concatenate([r["out"] for r in res.results], axis=0)
    if _trace:
        kernel.last_results = res
    return out
